# revision 9
# baseline (speedup 1.0000x reference)
"""Trainium2 Bass kernel for the contextual channel-attention transformer block.

Contract: kernel(**inputs) takes the FULL unsharded inputs
(x: (8,512,64,64) f32, Wq/Wk/Wv: (512,512) f32, gamma: (1,) f32) and
returns the FULL (8,512,64,64) f32 output.  Internally the batch is
data-parallel across 8 NeuronCores (one batch element per core).

Per-core algorithm (all bf16 matmuls, fp32 PSUM accumulation):
  Gx   = X @ X.T                     (C x C spatial Gram, 128 MMs)
  M3q  = Gx @ Wq.T, M3k = Gx @ Wk.T  (32 MMs)
  G^T  = Wk @ M3q  = (Q @ K.T).T     (16 MMs)
  |Q_c|^2 = diag(Wq Gx Wq.T) = colsum(Wq.T o M3q)   (cheap)
  cos -> col-max -> temperature -> softmax: free-axis ops on G^T[d, c]
  A^T  = Wv.T @ Msm^T = (Msm @ Wv).T (16 MMs)
  out  = A @ X                       (128 MMs)
  y    = x + (gamma / rowsum(Msm)) * out   (folded into per-partition scale)
"""

import os
import sys

for _p in ("/opt/trn_rl_repo", "/root/.axon_site/_ro/trn_rl_repo"):
    if os.path.isdir(_p) and _p not in sys.path:
        sys.path.insert(0, _p)

import ml_dtypes
import numpy as np

import concourse.bass as bass
import concourse.tile as tile
from concourse import bacc, bass_utils, mybir

# Problem constants (hardcoded; kernel.py must be self-contained).
B, C, HH, WW = 8, 512, 64, 64
N = HH * WW          # 4096 spatial positions
G = C // 128         # 4 channel groups of 128
N1 = N // 128        # 32 Gram chunks (128 spatial each)
NJ = N // 512        # 8 output chunks (512 spatial each)
EPS = 1e-6
INV_H = 4.0          # 1 / 0.25 temperature
FP32 = mybir.dt.float32
BF16 = mybir.dt.bfloat16

_CACHE = {}


def _warm(nc, pool, src_ap, k):
    scr = pool.tile([1, 1], mybir.dt.float32, tag="scr", bufs=2,
                    name=f"scr{k}")
    nc.tensor.matmul(scr[:], src_ap, src_ap, start=True, stop=True)


def _phase2_chunk(nc, tc, ps2, opool, at_sb, xh_t, fcols, xf_t, y_v, j):
    FP32 = mybir.dt.float32
    ADD = mybir.AluOpType.add
    Copy = mybir.ActivationFunctionType.Copy
    ofin = opool.tile([128, G, 512], FP32, tag="ofin", bufs=3,
                      name=f"ofin{j}")
    for cg in range(G):
        o_ps = ps2.tile([128, 512], FP32, tag="o_ps", bufs=6,
                        name=f"o_ps{j}_{cg}")
        for eg in range(G):
            nc.tensor.matmul(
                o_ps[:], at_sb[:, eg, cg * 128:(cg + 1) * 128],
                xh_t[j][:, eg, :],
                start=(eg == 0), stop=(eg == G - 1))
        osc = opool.tile([128, 512], FP32, tag="osc", bufs=4,
                         name=f"osc{j}_{cg}")
        nc.scalar.activation(osc[:], o_ps[:], Copy, scale=fcols[cg][:])
        nc.vector.tensor_tensor(ofin[:, cg, :], osc[:], xf_t[:, cg, :],
                                op=ADD)
    if j == NJ - 1:
        for cg in range(G):
            nc.sync.dma_start(y_v[:, cg, j * 512:(j + 1) * 512],
                              ofin[:, cg, :])
    else:
        nc.sync.dma_start(y_v[:, :, j * 512:(j + 1) * 512], ofin[:])


def _build_nc():
    nc = bacc.Bacc("TRN2", target_bir_lowering=False)

    xt_d = nc.dram_tensor("xt", [N, C], BF16, kind="ExternalInput")   # x^T
    xh_d = nc.dram_tensor("xh", [C, N], BF16, kind="ExternalInput")
    xf_d = nc.dram_tensor("xf", [C, N], FP32, kind="ExternalInput")
    wqt_d = nc.dram_tensor("wqt", [C, C], BF16, kind="ExternalInput")  # Wq^T
    wkt_d = nc.dram_tensor("wkt", [C, C], BF16, kind="ExternalInput")  # Wk^T
    wvo_d = nc.dram_tensor("wvo", [C, C], BF16, kind="ExternalInput")  # Wv
    gcol_d = nc.dram_tensor("gamma_col", [128, 1], FP32, kind="ExternalInput")
    ocol_d = nc.dram_tensor("ones_col", [128, 1], BF16, kind="ExternalInput")
    orow_d = nc.dram_tensor("ones_row", [1, C], BF16, kind="ExternalInput")
    y_d = nc.dram_tensor("y", [C, N], FP32, kind="ExternalOutput")

    xt_v = xt_d.ap().rearrange("(i p) c -> p i c", p=128)    # [128, N1, C]
    xh_v = xh_d.ap().rearrange("(g p) n -> p g n", p=128)    # [128, G, N]
    xf_v = xf_d.ap().rearrange("(g p) n -> p g n", p=128)
    wq_v = wqt_d.ap().rearrange("(g p) o -> p g o", p=128)   # [128, G, C]
    wk_v = wkt_d.ap().rearrange("(g p) o -> p g o", p=128)
    wv_v = wvo_d.ap().rearrange("(g p) o -> p g o", p=128)
    y_v = y_d.ap().rearrange("(g p) n -> p g n", p=128)

    MUL = mybir.AluOpType.mult
    ADD = mybir.AluOpType.add
    MIN = mybir.AluOpType.min
    AX = mybir.AxisListType.X
    Exp = mybir.ActivationFunctionType.Exp
    Ln = mybir.ActivationFunctionType.Ln
    Copy = mybir.ActivationFunctionType.Copy

    with tile.TileContext(nc) as tc:
        with (
            tc.tile_pool(name="consts", bufs=1) as cpool,
            tc.tile_pool(name="weights", bufs=1) as wpool,
            tc.tile_pool(name="xt", bufs=NJ) as xtpool,
            tc.tile_pool(name="xh", bufs=NJ) as xhpool,
            tc.tile_pool(name="gram", bufs=1) as gpool,
            tc.tile_pool(name="small", bufs=2) as spool,
            tc.tile_pool(name="mid", bufs=3) as mpool,
            tc.tile_pool(name="msm", bufs=1) as msmpool,
            tc.tile_pool(name="ph2", bufs=2) as p2pool,
            tc.tile_pool(name="outs", bufs=4) as opool,
        ):
            # ---- input DMAs (xt first: Gx depends only on it) ------------
            xt0 = []
            for i in range(G):
                t = xtpool.tile([128, 1, C], BF16, tag="xt0", bufs=G,
                                name=f"xt0_{i}")
                nc.sync.dma_start(t[:], xt_v[:, i:i + 1, :])
                xt0.append(t)
            xt_t = []
            for jj in range(1, NJ):
                t = xtpool.tile([128, G, C], BF16, tag="xt", bufs=NJ - 1,
                                name=f"xt{jj}")
                nc.sync.dma_start(t[:], xt_v[:, jj * G:(jj + 1) * G, :])
                xt_t.append(t)

            def xt_chunk(i):
                return xt0[i][:, 0, :] if i < G else xt_t[i // G - 1][:, i % G, :]

            ones_col = cpool.tile([128, 1], BF16, tag="ones_col")
            nc.sync.dma_start(ones_col[:], ocol_d.ap())
            ones_row = cpool.tile([1, C], BF16, tag="ones_row")
            nc.sync.dma_start(ones_row[:], orow_d.ap())
            gamma_col = cpool.tile([128, 1], FP32, tag="gamma_col")
            nc.sync.dma_start(gamma_col[:], gcol_d.ap())

            wq = wpool.tile([128, G, C], BF16, tag="wq")
            wk = wpool.tile([128, G, C], BF16, tag="wk")
            wv = wpool.tile([128, G, C], BF16, tag="wv")
            nc.sync.dma_start(wq[:], wq_v)
            nc.sync.dma_start(wk[:], wk_v)
            nc.sync.dma_start(wv[:], wv_v)

            xh_t = []
            for j in range(NJ):
                t = xhpool.tile([128, G, 512], BF16, tag="xh", name=f"xh{j}")
                nc.sync.dma_start(t[:], xh_v[:, :, j * 512:(j + 1) * 512])
                xh_t.append(t)

            # ---- Gx = X X^T  (PSUM-accumulated over 32 spatial chunks) ---
            gx_sb = gpool.tile([128, G, C], BF16, tag="gx_sb")
            with tc.tile_pool(name="psGx", bufs=1, space="PSUM") as psGx:
                gx_ps = [psGx.tile([128, C], FP32, tag="gx", bufs=G,
                                   name=f"gx{cg}") for cg in range(G)]
                for i in range(N1):
                    lhs_t = xt_chunk(i)
                    for cg in range(G):
                        nc.tensor.matmul(gx_ps[cg][:],
                                         lhs_t[:, cg * 128:(cg + 1) * 128],
                                         lhs_t[:],
                                         start=(i == 0), stop=(i == N1 - 1))
                for cg in range(G):
                    eng = nc.scalar.copy if cg % 2 else nc.vector.tensor_copy
                    eng(gx_sb[:, cg, :], gx_ps[cg][:])

            # ---- M3q = Gx Wq^T, M3k = Gx Wk^T ----------------------------
            m3q = gpool.tile([128, G, C], BF16, tag="m3q")
            m3k = gpool.tile([128, G, C], BF16, tag="m3k")
            with tc.tile_pool(name="psM3", bufs=1, space="PSUM") as psM3:
                for cg in range(G):
                    q_ps = psM3.tile([128, C], FP32, tag="m3q", bufs=G,
                                     name=f"m3q{cg}")
                    k_ps = psM3.tile([128, C], FP32, tag="m3k", bufs=G,
                                     name=f"m3k{cg}")
                    for g in range(G):
                        lhs = gx_sb[:, g, cg * 128:(cg + 1) * 128]
                        nc.tensor.matmul(q_ps[:], lhs, wq[:, g, :],
                                         start=(g == 0), stop=(g == G - 1))
                        nc.tensor.matmul(k_ps[:], lhs, wk[:, g, :],
                                         start=(g == 0), stop=(g == G - 1))
                    nc.scalar.copy(m3q[:, cg, :], q_ps[:])
                    nc.vector.tensor_copy(m3k[:, cg, :], k_ps[:])

            msm = msmpool.tile([128, G, C], BF16, tag="msm")
            at_sb = gpool.tile([128, G, C], BF16, tag="at_sb")
            fcols = []
            with tc.tile_pool(name="psN", bufs=1, space="PSUM") as psN:
                # ---- norms: |Q_c|^2 row, |K_d|^2 columns -----------------
                sqq = psN.tile([1, C], FP32, tag="sqq", name="sqq")
                sqk_ps = [psN.tile([128, 1], FP32, tag="sqk", bufs=G,
                                   name=f"sqk{d}") for d in range(G)]
                for g in range(G):
                    tq = mpool.tile([128, C], BF16, tag="tq")
                    nc.vector.tensor_tensor(tq[:], wq[:, g, :], m3q[:, g, :],
                                            op=MUL)
                    nc.tensor.matmul(sqq[:], ones_col[:], tq[:],
                                     start=(g == 0), stop=(g == G - 1))
                    tk = mpool.tile([128, C], BF16, tag="tk")
                    nc.vector.tensor_tensor(tk[:], wk[:, g, :],
                                            m3k[:, g, :], op=MUL)
                    for dg in range(G):
                        nc.tensor.matmul(sqk_ps[dg][:],
                                         tk[:, dg * 128:(dg + 1) * 128],
                                         ones_col[:],
                                         start=(g == 0), stop=(g == G - 1))

                # rq row (bf16, for broadcast matmul); rk columns (fp32)
                # 1/sqrt(s) = exp(-0.5*ln(s)); batch by ACT table set
                ln_q = spool.tile([1, C], FP32, tag="ln_q")
                nc.scalar.activation(ln_q[:], sqq[:], Ln)
                ln_ks = []
                for dg in range(G):
                    ln_k = spool.tile([128, 1], FP32, tag="ln_k", bufs=G,
                                      name=f"ln_k{dg}")
                    nc.scalar.activation(ln_k[:], sqk_ps[dg][:], Ln)
                    ln_ks.append(ln_k)
                rq_bf = spool.tile([1, C], BF16, tag="rq_bf")
                nc.scalar.activation(rq_bf[:], ln_q[:], Exp, scale=-0.5)
                rk_cols = []
                for dg in range(G):
                    rk = spool.tile([128, 1], FP32, tag="rk", bufs=G,
                                    name=f"rk{dg}")
                    nc.scalar.activation(rk[:], ln_ks[dg][:], Exp, scale=-0.5)
                    rk_cols.append(rk)

                _warm(nc, psN, ln_q[:, 0:1], 0)
                _warm(nc, psN, ln_q[:, 1:2], 1)

                bq_ps = psN.tile([128, C], FP32, tag="bq_ps", name="bq_ps")
                nc.tensor.matmul(bq_ps[:], ones_row[:, 0:128], rq_bf[:],
                                 start=True, stop=True)
                bq = mpool.tile([128, C], FP32, tag="bq", bufs=1)
                nc.scalar.copy(bq[:], bq_ps[:])

            with tc.tile_pool(name="psB", bufs=1, space="PSUM") as psB:
                # ---- G^T per d-group + transforms + A^T ------------------
                at_ps = [psB.tile([128, C], FP32, tag="at", bufs=G,
                                  name=f"at{eg}") for eg in range(G)]
                for dg in range(G):
                    g_ps = psB.tile([128, C], FP32, tag="g_ps", bufs=2,
                                    name=f"g_ps{dg}")
                    for g in range(G):
                        nc.tensor.matmul(g_ps[:],
                                         wk[:, g, dg * 128:(dg + 1) * 128],
                                         m3q[:, g, :],
                                         start=(g == 0), stop=(g == G - 1))
                    # cos = G^T * rq_c * rk_d
                    t1 = mpool.tile([128, C], FP32, tag="t1")
                    nc.vector.tensor_tensor(t1[:], g_ps[:], bq[:], op=MUL)
                    cosd = mpool.tile([128, C], FP32, tag="cosd")
                    nc.vector.tensor_scalar(cosd[:], t1[:], rk_cols[dg][:],
                                            None, op0=MUL)
                    _warm(nc, psB, t1[:, 0:1], 10 + dg * 3)
                    mn = spool.tile([128, 1], FP32, tag="mn")
                    nc.vector.tensor_reduce(mn[:], cosd[:], axis=AX, op=MIN)
                    _warm(nc, psB, cosd[:, 0:1], 11 + dg * 3)
                    den = spool.tile([128, 1], FP32, tag="den")
                    nc.vector.tensor_scalar(den[:], mn[:], -1.0, 1.0 + EPS,
                                            op0=MUL, op1=ADD)
                    r = spool.tile([128, 1], FP32, tag="r")
                    nc.vector.reciprocal(r[:], den[:])
                    sv = spool.tile([128, 1], FP32, tag="sv")
                    nc.vector.tensor_scalar(sv[:], r[:], INV_H, 0.0,
                                            op0=MUL, op1=ADD)
                    bv = spool.tile([128, 1], FP32, tag="bv")
                    nc.vector.tensor_scalar(bv[:], r[:], -INV_H, 1.0,
                                            op0=MUL, op1=ADD)
                    e = mpool.tile([128, C], BF16, tag="e")
                    se = spool.tile([128, 1], FP32, tag="se")
                    nc.scalar.activation(e[:], cosd[:], Exp,
                                         bias=bv[:], scale=sv[:],
                                         accum_out=se[:])
                    _warm(nc, psB, se[:], 12 + dg * 3)
                    rd = spool.tile([128, 1], FP32, tag="rd")
                    nc.vector.reciprocal(rd[:], se[:])
                    nc.vector.tensor_scalar(msm[:, dg, :], e[:], rd[:], None,
                                            op0=MUL)
                    # A^T accumulation: A^T = Wv^T-contracted over d
                    for eg in range(G):
                        nc.tensor.matmul(at_ps[eg][:],
                                         wv[:, dg, eg * 128:(eg + 1) * 128],
                                         msm[:, dg, :],
                                         start=(dg == 0), stop=(dg == G - 1))
                for eg in range(G):
                    nc.scalar.copy(at_sb[:, eg, :], at_ps[eg][:])

                # ---- row-L1 sums + final per-row scale -------------------
                s_list = []
                for cg in range(G):
                    s_ps = psB.tile([128, 1], FP32, tag="g_ps", bufs=2,
                                    name=f"s_ps{cg}")
                    for dg in range(G):
                        nc.tensor.matmul(
                            s_ps[:],
                            msm[:, dg, cg * 128:(cg + 1) * 128],
                            ones_col[:], start=(dg == 0), stop=(dg == G - 1))
                    s_list.append(s_ps)
                for cg in range(G):
                    speps = spool.tile([128, 1], FP32, tag="speps")
                    nc.vector.tensor_scalar(speps[:], s_list[cg][:],
                                            EPS, None, op0=ADD)
                    rs = spool.tile([128, 1], FP32, tag="rs")
                    nc.vector.reciprocal(rs[:], speps[:])
                    f = spool.tile([128, 1], FP32, tag="f", bufs=G,
                                   name=f"f{cg}")
                    nc.vector.tensor_tensor(f[:], rs[:], gamma_col[:], op=MUL)
                    fcols.append(f)

            # ---- phase 2: out = A X, scale, residual, store --------------
            with tc.tile_pool(name="ps2", bufs=1, space="PSUM") as ps2:
                xf_tiles = []
                for j in range(NJ):
                    xf_t = p2pool.tile([128, G, 512], FP32, tag="xf", bufs=3,
                                       name=f"xf{j}")
                    nc.sync.dma_start(xf_t[:],
                                      xf_v[:, :, j * 512:(j + 1) * 512])
                    xf_tiles.append(xf_t)
                    if j < 2:
                        continue          # prefetch two chunks ahead
                    _phase2_chunk(nc, tc, ps2, opool, at_sb, xh_t, fcols,
                                  xf_tiles[j - 2], y_v, j - 2)
                for j in (NJ - 2, NJ - 1):
                    _phase2_chunk(nc, tc, ps2, opool, at_sb, xh_t, fcols,
                                  xf_tiles[j], y_v, j)

    nc.compile()
    return nc


def _get_nc():
    if "nc" not in _CACHE:
        _CACHE["nc"] = _build_nc()
    return _CACHE["nc"]


def _make_in_maps(x, Wq, Wk, Wv, gamma):
    xb = np.ascontiguousarray(x.reshape(B, C, N).astype(np.float32))
    xb_h = xb.astype(ml_dtypes.bfloat16)
    xt_h = np.ascontiguousarray(xb_h.transpose(0, 2, 1))
    wqt = np.ascontiguousarray(Wq.T).astype(ml_dtypes.bfloat16)
    wkt = np.ascontiguousarray(Wk.T).astype(ml_dtypes.bfloat16)
    wvo = np.ascontiguousarray(Wv).astype(ml_dtypes.bfloat16)
    gcol = np.full((128, 1), float(np.asarray(gamma).reshape(-1)[0]),
                   np.float32)
    ocol = np.ones((128, 1), ml_dtypes.bfloat16)
    orow = np.ones((1, C), ml_dtypes.bfloat16)
    maps = []
    for i in range(B):
        maps.append({
            "xt": xt_h[i], "xh": xb_h[i], "xf": xb[i],
            "wqt": wqt, "wkt": wkt, "wvo": wvo,
            "gamma_col": gcol, "ones_col": ocol, "ones_row": orow,
        })
    return maps


def kernel(x, Wq, Wk, Wv, gamma, _trace=False, _trace_kwargs=None):
    nc = _get_nc()
    in_maps = _make_in_maps(np.asarray(x), np.asarray(Wq), np.asarray(Wk),
                            np.asarray(Wv), np.asarray(gamma))
    kwargs = {}
    if _trace:
        kwargs = dict(trace=True, **(_trace_kwargs or {}))
    res = bass_utils.run_bass_kernel_spmd(nc, in_maps,
                                          core_ids=list(range(B)), **kwargs)
    y = np.stack([res.results[i]["y"].reshape(C, HH, WW) for i in range(B)])
    if _trace:
        kernel._last_result = res
    return y.astype(np.float32)


# revision 10
# speedup vs baseline: 1.0821x; 1.0821x over previous
"""Trainium2 Bass kernel for the contextual channel-attention transformer block.

Contract: kernel(**inputs) takes the FULL unsharded inputs
(x: (8,512,64,64) f32, Wq/Wk/Wv: (512,512) f32, gamma: (1,) f32) and
returns the FULL (8,512,64,64) f32 output.  Internally the batch is
data-parallel across 8 NeuronCores (one batch element per core).

Per-core algorithm (all bf16 matmuls, fp32 PSUM accumulation):
  Gx   = X @ X.T                     (C x C spatial Gram, 128 MMs)
  M3q  = Gx @ Wq.T, M3k = Gx @ Wk.T  (32 MMs)
  G^T  = Wk @ M3q  = (Q @ K.T).T     (16 MMs)
  |Q_c|^2 = diag(Wq Gx Wq.T) = colsum(Wq.T o M3q)   (cheap)
  cos -> col-max -> temperature -> softmax: free-axis ops on G^T[d, c]
  A^T  = Wv.T @ Msm^T = (Msm @ Wv).T (16 MMs)
  out  = A @ X                       (128 MMs)
  y    = x + (gamma / rowsum(Msm)) * out   (folded into per-partition scale)
"""

import os
import sys

for _p in ("/opt/trn_rl_repo", "/root/.axon_site/_ro/trn_rl_repo"):
    if os.path.isdir(_p) and _p not in sys.path:
        sys.path.insert(0, _p)

import ml_dtypes
import numpy as np

import concourse.bass as bass
import concourse.tile as tile
from concourse import bacc, bass_utils, mybir

# Problem constants (hardcoded; kernel.py must be self-contained).
B, C, HH, WW = 8, 512, 64, 64
N = HH * WW          # 4096 spatial positions
G = C // 128         # 4 channel groups of 128
N1 = N // 128        # 32 Gram chunks (128 spatial each)
NJ = N // 512        # 8 output chunks (512 spatial each)
EPS = 1e-6
INV_H = 4.0          # 1 / 0.25 temperature
FP32 = mybir.dt.float32
BF16 = mybir.dt.bfloat16

_CACHE = {}


def _warm(nc, pool, src_ap, k):
    scr = pool.tile([1, 1], mybir.dt.float32, tag="scr", bufs=2,
                    name=f"scr{k}")
    nc.tensor.matmul(scr[:], src_ap, src_ap, start=True, stop=True)


def _phase2_chunk(nc, tc, ps2, opool, at_sb, xh_t, fcols, xf_t, y_v, j):
    FP32 = mybir.dt.float32
    ADD = mybir.AluOpType.add
    Copy = mybir.ActivationFunctionType.Copy
    ofin = opool.tile([128, G, 512], FP32, tag="ofin", bufs=3,
                      name=f"ofin{j}")
    for cg in range(G):
        o_ps = ps2.tile([128, 512], FP32, tag="o_ps", bufs=6,
                        name=f"o_ps{j}_{cg}")
        for eg in range(G):
            nc.tensor.matmul(
                o_ps[:], at_sb[:, eg, cg * 128:(cg + 1) * 128],
                xh_t[j][:, eg, :],
                start=(eg == 0), stop=(eg == G - 1))
        osc = opool.tile([128, 512], FP32, tag="osc", bufs=4,
                         name=f"osc{j}_{cg}")
        nc.scalar.activation(osc[:], o_ps[:], Copy, scale=fcols[cg][:])
        nc.vector.tensor_tensor(ofin[:, cg, :], osc[:], xf_t[:, cg, :],
                                op=ADD)
    if j == NJ - 1:
        for cg in range(G):
            nc.sync.dma_start(y_v[:, cg, j * 512:(j + 1) * 512],
                              ofin[:, cg, :])
    else:
        nc.sync.dma_start(y_v[:, :, j * 512:(j + 1) * 512], ofin[:])


def _build_nc():
    nc = bacc.Bacc("TRN2", target_bir_lowering=False)

    xt_d = nc.dram_tensor("xt", [N, C], BF16, kind="ExternalInput")   # x^T
    xh_d = nc.dram_tensor("xh", [C, N], BF16, kind="ExternalInput")
    xf_d = nc.dram_tensor("xf", [C, N], FP32, kind="ExternalInput")
    wqt_d = nc.dram_tensor("wqt", [C, C], BF16, kind="ExternalInput")  # Wq^T
    wkt_d = nc.dram_tensor("wkt", [C, C], BF16, kind="ExternalInput")  # Wk^T
    wvo_d = nc.dram_tensor("wvo", [C, C], BF16, kind="ExternalInput")  # Wv
    gcol_d = nc.dram_tensor("gamma_col", [128, 1], FP32, kind="ExternalInput")
    ocol_d = nc.dram_tensor("ones_col", [128, 1], BF16, kind="ExternalInput")
    orow_d = nc.dram_tensor("ones_row", [1, C], BF16, kind="ExternalInput")
    y_d = nc.dram_tensor("y", [C, N], FP32, kind="ExternalOutput")

    xt_v = xt_d.ap().rearrange("(i p) c -> p i c", p=128)    # [128, N1, C]
    xh_v = xh_d.ap().rearrange("(g p) n -> p g n", p=128)    # [128, G, N]
    xf_v = xf_d.ap().rearrange("(g p) n -> p g n", p=128)
    wq_v = wqt_d.ap().rearrange("(g p) o -> p g o", p=128)   # [128, G, C]
    wk_v = wkt_d.ap().rearrange("(g p) o -> p g o", p=128)
    wv_v = wvo_d.ap().rearrange("(g p) o -> p g o", p=128)
    y_v = y_d.ap().rearrange("(g p) n -> p g n", p=128)

    MUL = mybir.AluOpType.mult
    ADD = mybir.AluOpType.add
    MIN = mybir.AluOpType.min
    AX = mybir.AxisListType.X
    Exp = mybir.ActivationFunctionType.Exp
    Ln = mybir.ActivationFunctionType.Ln
    Copy = mybir.ActivationFunctionType.Copy

    with tile.TileContext(nc) as tc:
        with (
            tc.tile_pool(name="consts", bufs=1) as cpool,
            tc.tile_pool(name="weights", bufs=1) as wpool,
            tc.tile_pool(name="xt", bufs=NJ) as xtpool,
            tc.tile_pool(name="xh", bufs=NJ) as xhpool,
            tc.tile_pool(name="gram", bufs=1) as gpool,
            tc.tile_pool(name="small", bufs=2) as spool,
            tc.tile_pool(name="mid", bufs=3) as mpool,
            tc.tile_pool(name="msm", bufs=1) as msmpool,
            tc.tile_pool(name="ph2", bufs=2) as p2pool,
            tc.tile_pool(name="outs", bufs=4) as opool,
        ):
            # ---- input DMAs (xt first: Gx depends only on it) ------------
            xt0 = []
            for i in range(G):
                t = xtpool.tile([128, 1, C], BF16, tag="xt0", bufs=G,
                                name=f"xt0_{i}")
                nc.sync.dma_start(t[:], xt_v[:, i:i + 1, :])
                xt0.append(t)
            xt_t = []
            for jj in range(1, NJ):
                t = xtpool.tile([128, G, C], BF16, tag="xt", bufs=NJ - 1,
                                name=f"xt{jj}")
                nc.sync.dma_start(t[:], xt_v[:, jj * G:(jj + 1) * G, :])
                xt_t.append(t)

            def xt_chunk(i):
                return xt0[i][:, 0, :] if i < G else xt_t[i // G - 1][:, i % G, :]

            ones_col = cpool.tile([128, 1], BF16, tag="ones_col")
            nc.sync.dma_start(ones_col[:], ocol_d.ap())
            ones_row = cpool.tile([1, C], BF16, tag="ones_row")
            nc.sync.dma_start(ones_row[:], orow_d.ap())
            gamma_col = cpool.tile([128, 1], FP32, tag="gamma_col")
            nc.sync.dma_start(gamma_col[:], gcol_d.ap())

            wq = wpool.tile([128, G, C], BF16, tag="wq")
            wk = wpool.tile([128, G, C], BF16, tag="wk")
            wv = wpool.tile([128, G, C], BF16, tag="wv")
            nc.sync.dma_start(wq[:], wq_v)
            nc.sync.dma_start(wk[:], wk_v)
            nc.sync.dma_start(wv[:], wv_v)

            xh_t = []
            for j in range(NJ):
                t = xhpool.tile([128, G, 512], BF16, tag="xh", name=f"xh{j}")
                nc.sync.dma_start(t[:], xh_v[:, :, j * 512:(j + 1) * 512])
                xh_t.append(t)

            # ---- Gx = X X^T  (PSUM-accumulated over 32 spatial chunks) ---
            gx_sb = gpool.tile([128, G, C], BF16, tag="gx_sb")
            with tc.tile_pool(name="psGx", bufs=1, space="PSUM") as psGx:
                gx_ps = [psGx.tile([128, C], FP32, tag="gx", bufs=G,
                                   name=f"gx{cg}") for cg in range(G)]
                for i in range(N1):
                    lhs_t = xt_chunk(i)
                    for cg in range(G):
                        nc.tensor.matmul(gx_ps[cg][:],
                                         lhs_t[:, cg * 128:(cg + 1) * 128],
                                         lhs_t[:],
                                         start=(i == 0), stop=(i == N1 - 1))
                for cg in range(G):
                    eng = nc.scalar.copy if cg % 2 else nc.vector.tensor_copy
                    eng(gx_sb[:, cg, :], gx_ps[cg][:])

            # ---- M3q = Gx Wq^T, M3k = Gx Wk^T ----------------------------
            m3q = gpool.tile([128, G, C], BF16, tag="m3q")
            m3k = gpool.tile([128, G, C], BF16, tag="m3k")
            with tc.tile_pool(name="psM3", bufs=1, space="PSUM") as psM3:
                for cg in range(G):
                    q_ps = psM3.tile([128, C], FP32, tag="m3q", bufs=G,
                                     name=f"m3q{cg}")
                    k_ps = psM3.tile([128, C], FP32, tag="m3k", bufs=G,
                                     name=f"m3k{cg}")
                    for g in range(G):
                        lhs = gx_sb[:, g, cg * 128:(cg + 1) * 128]
                        nc.tensor.matmul(q_ps[:], lhs, wq[:, g, :],
                                         start=(g == 0), stop=(g == G - 1))
                        nc.tensor.matmul(k_ps[:], lhs, wk[:, g, :],
                                         start=(g == 0), stop=(g == G - 1))
                    nc.scalar.copy(m3q[:, cg, :], q_ps[:])
                    nc.vector.tensor_copy(m3k[:, cg, :], k_ps[:])

            msm = msmpool.tile([128, G, C], BF16, tag="msm")
            at_sb = gpool.tile([128, G, C], BF16, tag="at_sb")
            fcols = []
            with tc.tile_pool(name="psN", bufs=1, space="PSUM") as psN:
                # ---- norms: |Q_c|^2 row, |K_d|^2 columns -----------------
                sqq = psN.tile([1, C], FP32, tag="sqq", name="sqq")
                sqk_ps = [psN.tile([128, 1], FP32, tag="sqk", bufs=G,
                                   name=f"sqk{d}") for d in range(G)]
                for g in range(G):
                    tq = mpool.tile([128, C], BF16, tag="tq")
                    nc.vector.tensor_tensor(tq[:], wq[:, g, :], m3q[:, g, :],
                                            op=MUL)
                    nc.tensor.matmul(sqq[:], ones_col[:], tq[:],
                                     start=(g == 0), stop=(g == G - 1))
                    tk = mpool.tile([128, C], BF16, tag="tk")
                    nc.vector.tensor_tensor(tk[:], wk[:, g, :],
                                            m3k[:, g, :], op=MUL)
                    for dg in range(G):
                        nc.tensor.matmul(sqk_ps[dg][:],
                                         tk[:, dg * 128:(dg + 1) * 128],
                                         ones_col[:],
                                         start=(g == 0), stop=(g == G - 1))

                # rq row (bf16, for broadcast matmul); rk columns (fp32)
                # 1/sqrt(s) = exp(-0.5*ln(s)); batch by ACT table set
                ln_q = spool.tile([1, C], FP32, tag="ln_q")
                nc.scalar.activation(ln_q[:], sqq[:], Ln)
                ln_ks = []
                for dg in range(G):
                    ln_k = spool.tile([128, 1], FP32, tag="ln_k", bufs=G,
                                      name=f"ln_k{dg}")
                    nc.scalar.activation(ln_k[:], sqk_ps[dg][:], Ln)
                    ln_ks.append(ln_k)
                rq_bf = spool.tile([1, C], BF16, tag="rq_bf")
                nc.scalar.activation(rq_bf[:], ln_q[:], Exp, scale=-0.5)
                rk_cols = []
                for dg in range(G):
                    rk = spool.tile([128, 1], FP32, tag="rk", bufs=G,
                                    name=f"rk{dg}")
                    nc.scalar.activation(rk[:], ln_ks[dg][:], Exp, scale=-0.5)
                    rk_cols.append(rk)

                bq_ps = psN.tile([128, C], FP32, tag="bq_ps", name="bq_ps")
                nc.tensor.matmul(bq_ps[:], ones_row[:, 0:128], rq_bf[:],
                                 start=True, stop=True)
                bq = mpool.tile([128, C], FP32, tag="bq", bufs=1)
                nc.scalar.copy(bq[:], bq_ps[:])

            with tc.tile_pool(name="psB", bufs=1, space="PSUM") as psB:
                # ---- G^T per d-group + transforms + A^T ------------------
                at_ps = [psB.tile([128, C], FP32, tag="at", bufs=G,
                                  name=f"at{eg}") for eg in range(G)]
                for dg in range(G):
                    g_ps = psB.tile([128, C], FP32, tag="g_ps", bufs=2,
                                    name=f"g_ps{dg}")
                    for g in range(G):
                        nc.tensor.matmul(g_ps[:],
                                         wk[:, g, dg * 128:(dg + 1) * 128],
                                         m3q[:, g, :],
                                         start=(g == 0), stop=(g == G - 1))
                    # cos = G^T * rq_c * rk_d
                    t1 = mpool.tile([128, C], FP32, tag="t1")
                    nc.vector.tensor_tensor(t1[:], g_ps[:], bq[:], op=MUL)
                    cosd = mpool.tile([128, C], FP32, tag="cosd")
                    nc.vector.tensor_scalar(cosd[:], t1[:], rk_cols[dg][:],
                                            None, op0=MUL)
                    mn = spool.tile([128, 1], FP32, tag="mn")
                    nc.vector.tensor_reduce(mn[:], cosd[:], axis=AX, op=MIN)
                    den = spool.tile([128, 1], FP32, tag="den")
                    nc.vector.tensor_scalar(den[:], mn[:], -1.0, 1.0 + EPS,
                                            op0=MUL, op1=ADD)
                    r = spool.tile([128, 1], FP32, tag="r")
                    nc.vector.reciprocal(r[:], den[:])
                    sv = spool.tile([128, 1], FP32, tag="sv")
                    nc.vector.tensor_scalar(sv[:], r[:], INV_H, 0.0,
                                            op0=MUL, op1=ADD)
                    bv = spool.tile([128, 1], FP32, tag="bv")
                    nc.vector.tensor_scalar(bv[:], r[:], -INV_H, 1.0,
                                            op0=MUL, op1=ADD)
                    e = mpool.tile([128, C], BF16, tag="e")
                    se = spool.tile([128, 1], FP32, tag="se")
                    nc.scalar.activation(e[:], cosd[:], Exp,
                                         bias=bv[:], scale=sv[:],
                                         accum_out=se[:])
                    rd = spool.tile([128, 1], FP32, tag="rd")
                    nc.vector.reciprocal(rd[:], se[:])
                    nc.vector.tensor_scalar(msm[:, dg, :], e[:], rd[:], None,
                                            op0=MUL)
                    # A^T accumulation: A^T = Wv^T-contracted over d
                    for eg in range(G):
                        nc.tensor.matmul(at_ps[eg][:],
                                         wv[:, dg, eg * 128:(eg + 1) * 128],
                                         msm[:, dg, :],
                                         start=(dg == 0), stop=(dg == G - 1))
                for eg in range(G):
                    nc.scalar.copy(at_sb[:, eg, :], at_ps[eg][:])

                # ---- row-L1 sums + final per-row scale -------------------
                s_list = []
                for cg in range(G):
                    s_ps = psB.tile([128, 1], FP32, tag="g_ps", bufs=2,
                                    name=f"s_ps{cg}")
                    for dg in range(G):
                        nc.tensor.matmul(
                            s_ps[:],
                            msm[:, dg, cg * 128:(cg + 1) * 128],
                            ones_col[:], start=(dg == 0), stop=(dg == G - 1))
                    s_list.append(s_ps)
                for cg in range(G):
                    speps = spool.tile([128, 1], FP32, tag="speps")
                    nc.vector.tensor_scalar(speps[:], s_list[cg][:],
                                            EPS, None, op0=ADD)
                    rs = spool.tile([128, 1], FP32, tag="rs")
                    nc.vector.reciprocal(rs[:], speps[:])
                    f = spool.tile([128, 1], FP32, tag="f", bufs=G,
                                   name=f"f{cg}")
                    nc.vector.tensor_tensor(f[:], rs[:], gamma_col[:], op=MUL)
                    fcols.append(f)

            # ---- phase 2: out = A X, scale, residual, store --------------
            with tc.tile_pool(name="ps2", bufs=1, space="PSUM") as ps2:
                xf_tiles = []
                for j in range(NJ):
                    xf_t = p2pool.tile([128, G, 512], FP32, tag="xf", bufs=3,
                                       name=f"xf{j}")
                    nc.sync.dma_start(xf_t[:],
                                      xf_v[:, :, j * 512:(j + 1) * 512])
                    xf_tiles.append(xf_t)
                    if j < 2:
                        continue          # prefetch two chunks ahead
                    _phase2_chunk(nc, tc, ps2, opool, at_sb, xh_t, fcols,
                                  xf_tiles[j - 2], y_v, j - 2)
                for j in (NJ - 2, NJ - 1):
                    _phase2_chunk(nc, tc, ps2, opool, at_sb, xh_t, fcols,
                                  xf_tiles[j], y_v, j)

    nc.compile()
    return nc


def _get_nc():
    if "nc" not in _CACHE:
        _CACHE["nc"] = _build_nc()
    return _CACHE["nc"]


def _make_in_maps(x, Wq, Wk, Wv, gamma):
    xb = np.ascontiguousarray(x.reshape(B, C, N).astype(np.float32))
    xb_h = xb.astype(ml_dtypes.bfloat16)
    xt_h = np.ascontiguousarray(xb_h.transpose(0, 2, 1))
    wqt = np.ascontiguousarray(Wq.T).astype(ml_dtypes.bfloat16)
    wkt = np.ascontiguousarray(Wk.T).astype(ml_dtypes.bfloat16)
    wvo = np.ascontiguousarray(Wv).astype(ml_dtypes.bfloat16)
    gcol = np.full((128, 1), float(np.asarray(gamma).reshape(-1)[0]),
                   np.float32)
    ocol = np.ones((128, 1), ml_dtypes.bfloat16)
    orow = np.ones((1, C), ml_dtypes.bfloat16)
    maps = []
    for i in range(B):
        maps.append({
            "xt": xt_h[i], "xh": xb_h[i], "xf": xb[i],
            "wqt": wqt, "wkt": wkt, "wvo": wvo,
            "gamma_col": gcol, "ones_col": ocol, "ones_row": orow,
        })
    return maps


def kernel(x, Wq, Wk, Wv, gamma, _trace=False, _trace_kwargs=None):
    nc = _get_nc()
    in_maps = _make_in_maps(np.asarray(x), np.asarray(Wq), np.asarray(Wk),
                            np.asarray(Wv), np.asarray(gamma))
    kwargs = {}
    if _trace:
        kwargs = dict(trace=True, **(_trace_kwargs or {}))
    res = bass_utils.run_bass_kernel_spmd(nc, in_maps,
                                          core_ids=list(range(B)), **kwargs)
    y = np.stack([res.results[i]["y"].reshape(C, HH, WW) for i in range(B)])
    if _trace:
        kernel._last_result = res
    return y.astype(np.float32)
